# revision 1
# baseline (speedup 1.0000x reference)
"""Trainium2 Bass kernel for nn_MFF_38809324487316 (topk_masking).

Strategy (pure data parallel, batch sharded 16 -> 8 cores x 2 samples):
  Per sample, the whole gather/mean/1x1-conv pipeline is reformulated as
  one data-dependent [256,256] @ [256,6400] matmul:
    rows   0..127 : G (+I)   -> x1[pos_idx] + x1          (tmp1 gather, x1-add folded)
    row    128    : negmask/128 (+e128)                   (mean of negatives + x1)
    rows 129..255 : Wg = W_pos @ G + outer(w_last, negmask/128)   (the 1x1 conv)
  followed by BN+LeakyReLU (ACT engine, one instruction) + x1-add on the z rows.
  The 0/1 matrices are built on-device from the ECA channel scores via
  pairwise-comparison ranking (no sort, no data-dependent control flow).
"""

import sys

sys.path.insert(0, "/opt/trn_rl_repo")

import numpy as np

import concourse.bass as bass
import concourse.tile as tile
from concourse import mybir
from concourse.bass_utils import run_bass_kernel_spmd

B, C, H, W = 16, 256, 80, 80
HALF = C // 2          # 128
NPIX = H * W           # 6400
NCORES = 8
SPC = B // NCORES      # 2 samples per core
NT = 512               # matmul n-tile (one PSUM bank of f32)
BN_EPS = 1e-5
F32 = mybir.dt.float32


def build_nc(npix=NPIX, nsamp=SPC):
    nc = bass.Bass("TRN2", target_bir_lowering=False, debug=False)

    def din(name, shape):
        return nc.dram_tensor(name, shape, F32, kind="ExternalInput").ap()

    x0 = din("x0", [nsamp, C, npix])
    x1 = din("x1", [nsamp, C, npix])
    wposT = din("wposT", [HALF, HALF - 1])      # W_pos^T
    wlastb = din("wlastb", [HALF, HALF - 1])    # w_last broadcast over partitions
    bnA = din("bnA", [HALF, 1])                 # [0]=1, [p]=gamma/sqrt(var+eps) for row p-1
    bnB = din("bnB", [HALF, 1])                 # [0]=0, [p]=beta-mean*bnA
    s1c = din("s1c", [HALF, 1])                 # [0]=0, else 9/11   (lrelu decomp)
    s2c = din("s2c", [HALF, 1])                 # [0]=1, else 0.55
    ecaw = din("ecaw", [1, 5])
    id128 = din("id128", [HALF, HALF])
    tri = din("tri", [HALF, HALF])              # tri[k,j] = 1 if k<j
    onesm = din("onesm", [HALF, HALF])
    iota = din("iota", [HALF, HALF])            # iota[j,p] = p
    ones1r = din("ones1r", [1, HALF])
    out = nc.dram_tensor("out", [nsamp, 2 * C, npix], F32, kind="ExternalOutput").ap()

    ntl = []
    o = 0
    while o < npix:
        ntl.append((o, min(NT, npix - o)))
        o += NT

    from contextlib import ExitStack
    with tile.TileContext(nc) as tc, ExitStack() as st:
        consts = st.enter_context(tc.tile_pool(name="consts", bufs=1))
        xin = st.enter_context(tc.tile_pool(name="xin", bufs=1))
        lhp = st.enter_context(tc.tile_pool(name="lhp", bufs=4))
        gtp = st.enter_context(tc.tile_pool(name="gtp", bufs=2))
        misc = st.enter_context(tc.tile_pool(name="misc", bufs=2))
        obp = st.enter_context(tc.tile_pool(name="obp", bufs=2))
        pbig = st.enter_context(tc.tile_pool(name="pbig", bufs=4, space="PSUM"))
        pmisc = st.enter_context(tc.tile_pool(name="pmisc", bufs=3, space="PSUM"))
        if True:
            # ---- constants into SBUF ----
            c_id = consts.tile([HALF, HALF], F32)
            c_tri = consts.tile([HALF, HALF], F32)
            c_ones = consts.tile([HALF, HALF], F32)
            c_iota = consts.tile([HALF, HALF], F32)
            c_1r = consts.tile([1, HALF], F32)
            c_wposT = consts.tile([HALF, HALF - 1], F32)
            c_wlastb = consts.tile([HALF, HALF - 1], F32)
            c_bnA = consts.tile([HALF, 1], F32)
            c_bnB = consts.tile([HALF, 1], F32)
            c_s1 = consts.tile([HALF, 1], F32)
            c_s2 = consts.tile([HALF, 1], F32)
            c_ecaw = consts.tile([1, 5], F32)
            for t, d in (
                (c_id, id128), (c_tri, tri), (c_ones, onesm), (c_iota, iota),
                (c_1r, ones1r), (c_wposT, wposT), (c_wlastb, wlastb),
                (c_bnA, bnA), (c_bnB, bnB), (c_s1, s1c), (c_s2, s2c),
                (c_ecaw, ecaw),
            ):
                nc.gpsimd.dma_start(out=t, in_=d)

            # ---- x1 into SBUF (stays resident) ----
            X = [[None, None] for _ in range(nsamp)]
            for s in range(nsamp):
                for h in range(2):
                    t = xin.tile([HALF, npix], F32, tag=f"x_{s}_{h}")
                    nc.sync.dma_start(
                        out=t, in_=x1[s, h * HALF:(h + 1) * HALF, :])
                    X[s][h] = t

            # ---- phase A: channel sums -> row layout R [1, nsamp*(C+4)] ----
            R = misc.tile([1, nsamp * (C + 4)], F32, tag="R", bufs=1)
            nc.vector.memset(R, 0.0)
            for s in range(nsamp):
                for h in range(2):
                    sm = misc.tile([HALF, 1], F32, tag=f"sums_{s}_{h}", bufs=1)
                    nc.vector.reduce_sum(out=sm, in_=X[s][h],
                                         axis=mybir.AxisListType.X)
                    pr = pmisc.tile([1, HALF], F32, tag="mp")
                    nc.tensor.matmul(pr, sm, c_id, start=True, stop=True)
                    nc.scalar.copy(
                        out=R[0:1, s * (C + 4) + 2 + h * HALF:
                              s * (C + 4) + 2 + (h + 1) * HALF],
                        in_=pr)

            # ---- ECA conv k=5 along channels: Y [1, nsamp*C] ----
            Yr = misc.tile([1, nsamp, C], F32, tag="Y", bufs=1)
            Rr = R.rearrange("p (s c) -> p s c", s=nsamp)
            nc.vector.tensor_scalar(
                out=Yr, in0=Rr[:, :, 0:C], scalar1=c_ecaw[0:1, 0:1],
                scalar2=None, op0=mybir.AluOpType.mult)
            for k in range(1, 5):
                nc.vector.scalar_tensor_tensor(
                    out=Yr, in0=Rr[:, :, k:k + C], scalar=c_ecaw[0:1, k:k + 1],
                    in1=Yr, op0=mybir.AluOpType.mult, op1=mybir.AluOpType.add)

            # ---- per-sample: rank -> masks -> Lh ----
            LHS = [[None, None] for _ in range(nsamp)]
            for s in range(nsamp):
                pb = pmisc.tile([HALF, C], F32, tag="mp")
                nc.tensor.matmul(pb, c_1r, Yr[0:1, s, :], start=True, stop=True)
                m_sb, notm, negdiv = [], [], []
                for h in range(2):
                    pyc = pmisc.tile([HALF, 1], F32, tag="mp")
                    nc.tensor.matmul(
                        pyc, Yr[0:1, s, h * HALF:(h + 1) * HALF],
                        c_1r[0:1, 0:1], start=True, stop=True)
                    yc = misc.tile([HALF, 1], F32, tag="yc")
                    nc.scalar.copy(out=yc, in_=pyc)
                    cmp = misc.tile([HALF, C], F32, tag="cmp")
                    nc.vector.tensor_scalar(
                        out=cmp, in0=pb, scalar1=yc, scalar2=None,
                        op0=mybir.AluOpType.is_gt)
                    rankd = misc.tile([HALF, 1], F32, tag="rankd")
                    nc.vector.reduce_sum(out=rankd, in_=cmp,
                                         axis=mybir.AxisListType.X)
                    mh = misc.tile([HALF, 1], F32, tag="m")
                    nc.vector.tensor_scalar(
                        out=mh, in0=rankd, scalar1=float(HALF), scalar2=None,
                        op0=mybir.AluOpType.is_lt)
                    nh = misc.tile([HALF, 1], F32, tag="nm")
                    nc.vector.tensor_scalar(
                        out=nh, in0=rankd, scalar1=float(HALF), scalar2=None,
                        op0=mybir.AluOpType.is_ge)
                    nd = misc.tile([HALF, 1], F32, tag="nd")
                    nc.vector.tensor_scalar(
                        out=nd, in0=nh, scalar1=1.0 / HALF, scalar2=None,
                        op0=mybir.AluOpType.mult)
                    m_sb.append(mh); notm.append(nh); negdiv.append(nd)

                pr0 = pmisc.tile([HALF, 1], F32, tag="mp")
                nc.tensor.matmul(pr0, c_tri, m_sb[0], start=True, stop=True)
                pr1 = pmisc.tile([HALF, 1], F32, tag="mp")
                nc.tensor.matmul(pr1, c_ones, m_sb[0], start=True, stop=False)
                nc.tensor.matmul(pr1, c_tri, m_sb[1], start=False, stop=True)
                for h, prh in ((0, pr0), (1, pr1)):
                    rp = misc.tile([HALF, 1], F32, tag="rp")
                    nc.vector.scalar_tensor_tensor(
                        out=rp, in0=notm[h], scalar=256.0, in1=prh,
                        op0=mybir.AluOpType.mult, op1=mybir.AluOpType.add)
                    gt_sb = gtp.tile([HALF, HALF], F32, tag="gt")
                    nc.vector.tensor_scalar(
                        out=gt_sb, in0=c_iota, scalar1=rp, scalar2=None,
                        op0=mybir.AluOpType.is_equal)
                    pgm = pmisc.tile([HALF, HALF], F32, tag="mp")
                    nc.tensor.matmul(pgm, gt_sb, c_id, start=True, stop=True)
                    gm_sb = gtp.tile([HALF, HALF], F32, tag="gm")
                    nc.scalar.copy(out=gm_sb, in_=pgm)
                    pwg = pmisc.tile([HALF, HALF - 1], F32, tag="mp")
                    nc.tensor.matmul(pwg, gm_sb, c_wposT, start=True, stop=True)
                    lh = lhp.tile([HALF, C], F32, tag="lh")
                    if h == 0:
                        nc.vector.tensor_add(
                            out=lh[:, 0:HALF], in0=gt_sb, in1=c_id)
                    else:
                        nc.vector.tensor_copy(out=lh[:, 0:HALF], in_=gt_sb)
                    nc.vector.tensor_copy(
                        out=lh[:, HALF:HALF + 1], in_=negdiv[h])
                    nc.vector.scalar_tensor_tensor(
                        out=lh[:, HALF + 1:C], in0=c_wlastb, scalar=negdiv[h],
                        in1=pwg, op0=mybir.AluOpType.mult,
                        op1=mybir.AluOpType.add)
                    LHS[s][h] = lh
                # NOTE: no identity fold for inner row 128 — the mh1 epilogue
                # adds x1 for all 128 partitions (incl. the mean row at p=0).

            # ---- big matmuls + epilogue + stores ----
            for s in range(nsamp):
                for mh in range(2):
                    ob = obp.tile([HALF, npix], F32, tag="ob")
                    for (n0, nsz) in ntl:
                        ps = pbig.tile([HALF, NT], F32, tag="pb")
                        nc.tensor.matmul(
                            ps[:, :nsz], LHS[s][0][:, mh * HALF:(mh + 1) * HALF],
                            X[s][0][:, n0:n0 + nsz], start=True, stop=False)
                        nc.tensor.matmul(
                            ps[:, :nsz], LHS[s][1][:, mh * HALF:(mh + 1) * HALF],
                            X[s][1][:, n0:n0 + nsz], start=False, stop=True)
                        if mh == 0:
                            nc.scalar.copy(out=ob[:, n0:n0 + nsz], in_=ps[:, :nsz])
                        else:
                            # lrelu(q) = 0.55q + 0.45|q|; p=0 (mean row) passes
                            # through via the per-partition scalar masks.
                            tq = misc.tile([HALF, NT], F32, tag="tq", bufs=3)
                            ta = misc.tile([HALF, NT], F32, tag="ta", bufs=3)
                            nc.vector.tensor_scalar(
                                out=tq[:, :nsz], in0=ps[:, :nsz],
                                scalar1=c_bnA, scalar2=c_bnB,
                                op0=mybir.AluOpType.mult,
                                op1=mybir.AluOpType.add)
                            nc.scalar.activation(
                                out=ta[:, :nsz], in_=ps[:, :nsz],
                                func=mybir.ActivationFunctionType.Abs,
                                bias=c_bnB, scale=c_bnA)
                            nc.vector.scalar_tensor_tensor(
                                out=tq[:, :nsz], in0=ta[:, :nsz], scalar=c_s1,
                                in1=tq[:, :nsz], op0=mybir.AluOpType.mult,
                                op1=mybir.AluOpType.add)
                            nc.vector.scalar_tensor_tensor(
                                out=ob[:, n0:n0 + nsz], in0=tq[:, :nsz],
                                scalar=c_s2, in1=X[s][1][:, n0:n0 + nsz],
                                op0=mybir.AluOpType.mult,
                                op1=mybir.AluOpType.add)
                    nc.sync.dma_start(
                        out=out[s, C + mh * HALF:C + (mh + 1) * HALF, :], in_=ob)
                # x0 passthrough DRAM->DRAM on the SWDGE ring
                nc.gpsimd.dma_start(out=out[s, 0:C, :], in_=x0[s, :, :])
    return nc


def _split_multiwait_drains(nc):
    """This container's walrus rejects >1 sync-wait on one instruction
    ("Too many sync wait commands" in setupSyncWait). Tile's kernel-tail
    Drain carries one wait per outstanding semaphore — split it into a
    chain of single-wait Drains."""
    for fn in nc.m.functions:
        for blk in fn.blocks:
            insts = list(blk.instructions)
            changed = False
            out = []
            for inst in insts:
                si = getattr(inst, "sync_info", None)
                waits = list(si.on_wait) if (si and si.on_wait) else []
                if len(waits) > 1:
                    for j, w in enumerate(waits[:-1]):
                        nd = mybir.InstEventSemaphore(
                            name=f"{inst.name}-sw{j}", ins=[], outs=[])
                        nd.engine = inst.engine
                        nd.sync_info = mybir.SyncInfo(
                            on_wait=[w], on_update=[])
                        out.append(nd)
                    si.on_wait = [waits[-1]]
                    changed = True
                out.append(inst)
            if changed:
                blk.instructions = out
    return nc


def host_consts(conv_w, bn_gamma, bn_beta, bn_mean, bn_var, eca_w):
    conv_w = np.asarray(conv_w, np.float32)
    a = (np.asarray(bn_gamma, np.float64)
         / np.sqrt(np.asarray(bn_var, np.float64) + BN_EPS))
    bnA = np.zeros((HALF, 1), np.float32)
    bnB = np.zeros((HALF, 1), np.float32)
    bnA[0, 0] = 1.0
    bnA[1:, 0] = a.astype(np.float32)
    bnB[1:, 0] = (np.asarray(bn_beta, np.float64)
                  - np.asarray(bn_mean, np.float64) * a).astype(np.float32)
    s1c = np.full((HALF, 1), 0.45 / 0.55, np.float32); s1c[0, 0] = 0.0
    s2c = np.full((HALF, 1), 0.55, np.float32); s2c[0, 0] = 1.0
    return {
        "s1c": s1c,
        "s2c": s2c,
        "wposT": np.ascontiguousarray(conv_w[:, :HALF].T),
        "wlastb": np.ascontiguousarray(
            np.tile(conv_w[:, HALF][None, :], (HALF, 1))),
        "bnA": bnA,
        "bnB": bnB,
        "ecaw": np.asarray(eca_w, np.float32).reshape(1, 5),
        "id128": np.eye(HALF, dtype=np.float32),
        "tri": np.triu(np.ones((HALF, HALF), np.float32), 1),
        "onesm": np.ones((HALF, HALF), np.float32),
        "iota": np.tile(np.arange(HALF, dtype=np.float32), (HALF, 1)),
        "ones1r": np.ones((1, HALF), np.float32),
    }


def kernel(x0, x1, eca_w, conv_w, bn_gamma, bn_beta, bn_mean, bn_var,
           _trace=False):
    x0 = np.asarray(x0, np.float32).reshape(B, C, NPIX)
    x1 = np.asarray(x1, np.float32).reshape(B, C, NPIX)
    cst = host_consts(conv_w, bn_gamma, bn_beta, bn_mean, bn_var, eca_w)
    nc = _split_multiwait_drains(build_nc())
    in_maps = []
    for c in range(NCORES):
        m = dict(cst)
        m["x0"] = np.ascontiguousarray(x0[c * SPC:(c + 1) * SPC])
        m["x1"] = np.ascontiguousarray(x1[c * SPC:(c + 1) * SPC])
        in_maps.append(m)
    res = run_bass_kernel_spmd(nc, in_maps, list(range(NCORES)), trace=False)
    out = np.concatenate([res.results[c]["out"] for c in range(NCORES)], axis=0)
    out = out.reshape(B, 2 * C, H, W)
    return out


def bench(x0, x1, eca_w, conv_w, bn_gamma, bn_beta, bn_mean, bn_var,
          iters=10, warmup=3):
    """Returns (output, median_exec_seconds). Times a pre-jitted SPMD call
    with inputs already on device (no donation; kernel writes all outputs)."""
    import time
    import jax
    from jax.sharding import Mesh, PartitionSpec
    from jax.experimental.shard_map import shard_map
    from concourse import bass2jax
    from concourse import mybir as _mb

    x0 = np.asarray(x0, np.float32).reshape(B, C, NPIX)
    x1 = np.asarray(x1, np.float32).reshape(B, C, NPIX)
    cst = host_consts(conv_w, bn_gamma, bn_beta, bn_mean, bn_var, eca_w)
    nc = _split_multiwait_drains(build_nc())
    bass2jax.install_neuronx_cc_hook()

    # Mirror bass2jax.run_bass_via_pjrt exactly (donated zero output buffers,
    # partition_id appended last), but keep the jitted fn for repeat timing.
    import jax.numpy as jnp
    pid_name = (nc.partition_id_tensor.name
                if nc.partition_id_tensor else None)
    in_names, out_names, out_avals, zero_shapes = [], [], [], []
    for alloc in nc.m.functions[0].allocations:
        if not isinstance(alloc, _mb.MemoryLocationSet):
            continue
        name = alloc.memorylocations[0].name
        if alloc.kind == "ExternalInput":
            if name != pid_name:
                in_names.append(name)
        elif alloc.kind == "ExternalOutput":
            out_names.append(name)
            shape = tuple(alloc.tensor_shape)
            dtype = _mb.dt.np(alloc.dtype)
            out_avals.append(jax.core.ShapedArray(shape, dtype))
            zero_shapes.append((shape, dtype))
    n_params = len(in_names)
    n_outs = len(out_names)
    all_names = list(in_names) + list(out_names) + (
        [pid_name] if pid_name else [])
    donate = tuple(range(n_params, n_params + n_outs))

    def _body(*args):
        operands = list(args)
        if pid_name:
            operands.append(bass2jax.partition_id_tensor())
        outs = bass2jax._bass_exec_p.bind(
            *operands, out_avals=tuple(out_avals), in_names=tuple(all_names),
            out_names=tuple(out_names), lowering_input_output_aliases=(),
            sim_require_finite=True, sim_require_nnan=True, nc=nc)
        return tuple(outs)

    devices = jax.devices()[:NCORES]
    mesh = Mesh(np.asarray(devices), ("core",))
    fn = jax.jit(shard_map(
        _body, mesh=mesh,
        in_specs=(PartitionSpec("core"),) * (n_params + n_outs),
        out_specs=(PartitionSpec("core"),) * n_outs,
        check_rep=False), donate_argnums=donate, keep_unused=True)

    per_core = []
    for c in range(NCORES):
        m = dict(cst)
        m["x0"] = np.ascontiguousarray(x0[c * SPC:(c + 1) * SPC])
        m["x1"] = np.ascontiguousarray(x1[c * SPC:(c + 1) * SPC])
        per_core.append(m)
    concat_in = [np.concatenate([per_core[c][n] for c in range(NCORES)], axis=0)
                 for n in in_names]
    concat_zeros = [np.zeros((NCORES * sh[0], *sh[1:]), dt)
                    for (sh, dt) in zero_shapes]

    # first call: numpy operands, exactly like run_bass_via_pjrt (compiles)
    outs = fn(*concat_in, *concat_zeros)
    jax.block_until_ready(outs)
    oidx = out_names.index("out")
    full = np.asarray(outs[oidx]).reshape(NCORES, SPC, 2 * C, NPIX)
    result = full.reshape(B, 2 * C, H, W)

    # timed iterations: inputs pre-staged on device; fresh on-device zeros
    # per call (donated). Transfers stay outside the timed window.
    sharding = jax.sharding.NamedSharding(mesh, PartitionSpec("core"))
    dev_in = [jax.device_put(a, sharding) for a in concat_in]
    zfn = jax.jit(
        lambda: tuple(jnp.zeros((NCORES * sh[0], *sh[1:]), dt)
                      for (sh, dt) in zero_shapes),
        out_shardings=(sharding,) * n_outs)
    times = []
    for i in range(warmup + iters):
        z = zfn()
        jax.block_until_ready(z)
        t0 = time.perf_counter()
        r = fn(*dev_in, *z)
        jax.block_until_ready(r)
        dt_s = time.perf_counter() - t0
        if i >= warmup:
            times.append(dt_s)
        del r
    times.sort()
    med = times[len(times) // 2]
    return result, med, times



# revision 2
# speedup vs baseline: 389.0748x; 389.0748x over previous
"""Trainium2 Bass kernel for nn_MFF_38809324487316 (topk_masking).

Pure data parallel: batch dim 16 -> 8 cores x 2 samples; the tiny ECA/conv/BN
params are replicated (folded into one packed constant block per core).

Per sample, the whole top-k gather / mean / 1x1-conv pipeline is one
data-dependent [256,256] @ [256,6400] matmul:
  rows   0..127 : G            -> x1[pos_idx]            (tmp1 gather rows)
  row    128    : negmask/128  -> mean of negatives      (tmp1 mean row)
  rows 129..255 : W_pos @ G + outer(w_last, negmask/128) (the 1x1 conv)
followed by BN + LeakyReLU (ACT affine + one max-based VE op; the mean row
passes through via per-partition alpha[0]=1) and a +x1 add.

The data-dependent 0/1 matrices are built on-device from the ECA channel
scores with no sort and no data-dependent control flow:
  - channel scores y for BOTH samples via two accumulating PE matmuls
    against a host-built banded matrix (GAP + ECA conv fused; sigmoid
    dropped - it is monotone so the ranking is unchanged),
  - ranks for all 4 (sample, half) pairs via fused compare+row-sum
    (tensor_scalar with accum_out),
  - gather matrix G via iota == position equality.

All big transfers ride ONE HWDGE queue (nc.sync) - measured faster on HW
than splitting across queues; the x0 passthrough is DRAM->DRAM after each
sample's stores. Per-iteration HW time ~178 us/core vs a measured pure-DMA
floor of ~165 us for the same traffic (52.4 MB HBM per core per iteration).
"""

import sys

sys.path.insert(0, "/opt/trn_rl_repo")

import numpy as np

import concourse.bass as bass
import concourse.tile as tile
from concourse import mybir
from concourse.bass_utils import run_bass_kernel_spmd

B, C, H, W = 16, 256, 80, 80
HALF = C // 2           # 128
NPIX = H * W            # 6400
NCORES = 8
SPC = B // NCORES       # 2 samples per core
NT = 512                # matmul n-tile (one PSUM bank of f32)
GRP = 1024              # epilogue group (2 PSUM banks)
BN_EPS = 1e-5
F32 = mybir.dt.float32

# cblk column offsets
O_ID = 0
O_TRI = 128
O_ONES = 256
O_IOTA = 384
O_B0 = 512
O_B1 = 768
O_WPT = 1024
O_WLB = 1151
O_BNA = 1278
O_BNB = 1279
O_ALP = 1280
O_SEL = 1281
CBLK_W = 1537


def host_consts(conv_w, bn_gamma, bn_beta, bn_mean, bn_var, eca_w):
    w = np.asarray(eca_w, np.float64).reshape(5)
    conv_w = np.asarray(conv_w, np.float64)          # [127, 129]
    id128 = np.eye(HALF)
    tri = np.triu(np.ones((HALF, HALF)), 1)          # tri[k, j] = 1 iff k < j
    ones = np.ones((HALF, HALF))
    iota = np.tile(np.arange(HALF, dtype=np.float64), (HALF, 1))
    Bm = np.zeros((2, HALF, C))
    for h in range(2):
        for k in range(HALF):
            c = h * HALF + k
            for t in range(5):
                cp = c - t + 2
                if 0 <= cp < C:
                    Bm[h, k, cp] = w[t]
    wposT = conv_w[:, :HALF].T                        # [128, 127]
    wlastb = np.tile(conv_w[:, HALF][None, :], (HALF, 1))
    a = np.asarray(bn_gamma, np.float64) / np.sqrt(
        np.asarray(bn_var, np.float64) + BN_EPS)
    bnA = np.zeros((HALF, 1)); bnA[0, 0] = 1.0; bnA[1:, 0] = a
    bnB = np.zeros((HALF, 1))
    bnB[1:, 0] = (np.asarray(bn_beta, np.float64)
                  - np.asarray(bn_mean, np.float64) * a)
    alpha = np.full((HALF, 1), 0.1); alpha[0, 0] = 1.0
    sel = np.zeros((HALF, 2 * HALF))
    sel[0, 0:HALF] = 1.0
    sel[1, HALF:2 * HALF] = 1.0
    cblk = np.concatenate(
        [id128, tri, ones, iota, Bm[0], Bm[1], wposT, wlastb, bnA, bnB, alpha,
         sel], axis=1).astype(np.float32)
    assert cblk.shape == (HALF, CBLK_W)
    return {"cblk": cblk}


def build_nc(reps=1, npix=NPIX, nsamp=SPC, mh0_engine="vector"):
    nc = bass.Bass("TRN2", target_bir_lowering=False, debug=False)

    x0 = nc.dram_tensor("x0", [nsamp, C, npix], F32, kind="ExternalInput").ap()
    x1 = nc.dram_tensor("x1", [nsamp, C, npix], F32, kind="ExternalInput").ap()
    cbd = nc.dram_tensor("cblk", [HALF, CBLK_W], F32, kind="ExternalInput").ap()
    out = nc.dram_tensor("out", [nsamp, 2 * C, npix], F32,
                         kind="ExternalOutput").ap()

    AL = mybir.AluOpType
    from contextlib import ExitStack
    with tile.TileContext(nc) as tc, ExitStack() as st:
        consts = st.enter_context(tc.tile_pool(name="consts", bufs=1))
        xin = st.enter_context(tc.tile_pool(name="xin", bufs=1))
        lhp = st.enter_context(tc.tile_pool(name="lhp", bufs=1))
        misc = st.enter_context(tc.tile_pool(name="misc", bufs=1))
        epi = st.enter_context(tc.tile_pool(name="epi", bufs=2))
        obp = st.enter_context(tc.tile_pool(name="obp", bufs=2))
        prk = st.enter_context(tc.tile_pool(name="prk", bufs=3, space="PSUM"))
        pbig = st.enter_context(tc.tile_pool(name="pbig", bufs=2, space="PSUM"))

        cb = consts.tile([HALF, CBLK_W], F32)
        nc.sync.dma_start(out=cb, in_=cbd)
        c_id = cb[:, O_ID:O_ID + 128]
        c_id2 = cb[0:2, O_ID:O_ID + 2]
        c_tri = cb[:, O_TRI:O_TRI + 128]
        c_ones = cb[:, O_ONES:O_ONES + 128]
        c_iota = cb[:, O_IOTA:O_IOTA + 128]
        c_B = [cb[:, O_B0:O_B0 + C], cb[:, O_B1:O_B1 + C]]
        c_wposT = cb[:, O_WPT:O_WPT + 127]
        c_wlastb = cb[:, O_WLB:O_WLB + 127]
        c_bnA = cb[:, O_BNA:O_BNA + 1]
        c_bnB = cb[:, O_BNB:O_BNB + 1]
        c_alpha = cb[:, O_ALP:O_ALP + 1]

        for rep in range(reps):
            # ---- loads ----
            X = [[None, None] for _ in range(nsamp)]
            for s in range(nsamp):
                for h in range(2):
                    t = xin.tile([HALF, npix], F32, tag=f"x_{s}_{h}")
                    nc.sync.dma_start(out=t,
                                      in_=x1[s, h * HALF:(h + 1) * HALF, :])
                    X[s][h] = t

            # ---- channel sums: SM[:, h*2+s] ----
            SM = misc.tile([HALF, 4], F32, tag="SM")
            for s in range(nsamp):
                for h in range(2):
                    nc.vector.reduce_sum(out=SM[:, h * 2 + s:h * 2 + s + 1],
                                         in_=X[s][h], axis=mybir.AxisListType.X)

            # ---- scores y (GAP+ECA fused): Y2 [2, 256] = sum_h SM_h^T @ B_h ----
            Y2 = prk.tile([2, C], F32, tag="mp")
            nc.tensor.matmul(Y2, SM[:, 0:2], c_B[0], start=True, stop=False)
            nc.tensor.matmul(Y2, SM[:, 2:4], c_B[1], start=False, stop=True)
            y_sb = misc.tile([2, C], F32, tag="ysb")
            nc.vector.tensor_copy(out=y_sb, in_=Y2)

            # ---- yT [128, 4]: y with channel-as-partition ----
            pyT = prk.tile([HALF, 4], F32, tag="mp")
            for h in range(2):
                nc.tensor.matmul(pyT[:, h * 2:h * 2 + 2],
                                 y_sb[:, h * HALF:(h + 1) * HALF], c_id2,
                                 start=True, stop=True)
            ycT = misc.tile([HALF, 4], F32, tag="ycT")
            nc.vector.tensor_copy(out=ycT, in_=pyT)

            # ---- broadcast y along partitions: pbY [128, 512] ----
            pbY = prk.tile([HALF, 2 * C], F32, tag="mp")
            for s in range(nsamp):
                nc.tensor.matmul(pbY[:, s * C:(s + 1) * C],
                                 cb[0:2, O_SEL + s * HALF:O_SEL + (s + 1) * HALF],
                                 y_sb, start=True, stop=True)

            # ---- ranks RD[:, h*2+s] = #{c' : y[c'] > y[c]} ----
            RD = misc.tile([HALF, 4], F32, tag="RD")
            for s in range(nsamp):
                for h in range(2):
                    junk = misc.tile([HALF, C], F32, tag="junk", bufs=2)
                    nc.vector.tensor_scalar(
                        out=junk, in0=pbY[:, s * C:(s + 1) * C],
                        scalar1=ycT[:, h * 2 + s:h * 2 + s + 1], scalar2=None,
                        op0=AL.is_gt, op1=AL.add,
                        accum_out=RD[:, h * 2 + s:h * 2 + s + 1])

            # ---- masks ----
            M = misc.tile([HALF, 4], F32, tag="M")
            ND = misc.tile([HALF, 4], F32, tag="ND")
            nc.vector.tensor_scalar(out=M, in0=RD, scalar1=float(HALF),
                                    scalar2=None, op0=AL.is_lt)
            nc.vector.tensor_scalar(out=ND, in0=RD, scalar1=float(HALF),
                                    scalar2=1.0 / HALF, op0=AL.is_ge,
                                    op1=AL.mult)

            # ---- positions P; RP = 32768*ND + P ----
            P = prk.tile([HALF, 4], F32, tag="mp")
            nc.tensor.matmul(P[:, 0:2], c_tri, M[:, 0:2], start=True, stop=True)
            nc.tensor.matmul(P[:, 2:4], c_tri, M[:, 2:4], start=True, stop=False)
            nc.tensor.matmul(P[:, 2:4], c_ones, M[:, 0:2], start=False,
                             stop=True)
            RP = misc.tile([HALF, 4], F32, tag="RP")
            nc.vector.scalar_tensor_tensor(out=RP, in0=ND, scalar=32768.0,
                                           in1=P, op0=AL.mult, op1=AL.add)

            # ---- G columns + negdiv column of LHS ----
            LHS = [[None, None] for _ in range(nsamp)]
            for s in range(nsamp):
                for h in range(2):
                    lh = lhp.tile([HALF, C], F32, tag=f"lh_{s}_{h}")
                    LHS[s][h] = lh
                    nc.vector.tensor_scalar(
                        out=lh[:, 0:HALF], in0=c_iota,
                        scalar1=RP[:, h * 2 + s:h * 2 + s + 1], scalar2=None,
                        op0=AL.is_equal)
                    nc.vector.tensor_copy(out=lh[:, HALF:HALF + 1],
                                          in_=ND[:, h * 2 + s:h * 2 + s + 1])

            # ---- W columns: transpose G, multiply by W_pos^T, assemble ----
            sh_pairs = [(s, h) for s in range(nsamp) for h in range(2)]
            pgm = prk.tile([HALF, 4 * HALF], F32, tag="mp")
            for i, (s, h) in enumerate(sh_pairs):
                nc.tensor.matmul(pgm[:, i * HALF:(i + 1) * HALF],
                                 LHS[s][h][:, 0:HALF], c_id,
                                 start=True, stop=True)
            gm_all = misc.tile([HALF, 4 * HALF], F32, tag="gm")
            nc.vector.tensor_copy(out=gm_all, in_=pgm)
            pwg = prk.tile([HALF, 4 * HALF], F32, tag="mp")
            for i, (s, h) in enumerate(sh_pairs):
                nc.tensor.matmul(pwg[:, i * HALF:i * HALF + 127],
                                 gm_all[:, i * HALF:(i + 1) * HALF], c_wposT,
                                 start=True, stop=True)
            for i, (s, h) in enumerate(sh_pairs):
                nc.vector.scalar_tensor_tensor(
                    out=LHS[s][h][:, HALF + 1:C], in0=c_wlastb,
                    scalar=ND[:, h * 2 + s:h * 2 + s + 1],
                    in1=pwg[:, i * HALF:i * HALF + 127],
                    op0=AL.mult, op1=AL.add)

            # ---- big matmuls + epilogue + stores ----
            grps = []
            g0 = 0
            while g0 < npix:
                grps.append((g0, min(GRP, npix - g0)))
                g0 += GRP
            for s in range(nsamp):
                for mh in range(2):
                    ob = obp.tile([HALF, npix], F32, tag="ob")
                    for (g0, gsz) in grps:
                        ps = pbig.tile([HALF, GRP], F32, tag="pb")
                        n0 = 0
                        while n0 < gsz:
                            nsz = min(NT, gsz - n0)
                            for h in range(2):
                                nc.tensor.matmul(
                                    ps[:, n0:n0 + nsz],
                                    LHS[s][h][:, mh * HALF:(mh + 1) * HALF],
                                    X[s][h][:, g0 + n0:g0 + n0 + nsz],
                                    start=(h == 0), stop=(h == 1))
                            n0 += nsz
                        if mh == 0:
                            getattr(nc, mh0_engine).tensor_add(
                                out=ob[:, g0:g0 + gsz], in0=ps[:, :gsz],
                                in1=X[s][0][:, g0:g0 + gsz])
                        else:
                            q = epi.tile([HALF, GRP], F32, tag="q")
                            nc.scalar.activation(
                                out=q[:, :gsz], in_=ps[:, :gsz],
                                func=mybir.ActivationFunctionType.Identity,
                                bias=c_bnB, scale=c_bnA)
                            m = epi.tile([HALF, GRP], F32, tag="m")
                            nc.vector.scalar_tensor_tensor(
                                out=m[:, :gsz], in0=q[:, :gsz], scalar=c_alpha,
                                in1=q[:, :gsz], op0=AL.mult, op1=AL.max)
                            nc.vector.tensor_add(out=ob[:, g0:g0 + gsz],
                                                 in0=m[:, :gsz],
                                                 in1=X[s][1][:, g0:g0 + gsz])
                    nc.sync.dma_start(
                        out=out[s, C + mh * HALF:C + (mh + 1) * HALF, :],
                        in_=ob)
                # x0 passthrough DRAM->DRAM, after this sample's stores
                nc.sync.dma_start(out=out[s, 0:C, :], in_=x0[s, :, :])
    return nc


def _split_multiwait_drains(nc):
    """This container's walrus rejects >1 sync-wait on one instruction -
    split Tile's kernel-tail multi-wait Drains into single-wait chains."""
    for fn in nc.m.functions:
        for blk in fn.blocks:
            insts = list(blk.instructions)
            changed = False
            outl = []
            for inst in insts:
                si = getattr(inst, "sync_info", None)
                waits = list(si.on_wait) if (si and si.on_wait) else []
                if len(waits) > 1:
                    for j, w in enumerate(waits[:-1]):
                        nd = mybir.InstEventSemaphore(
                            name=f"{inst.name}-sw{j}", ins=[], outs=[])
                        nd.engine = inst.engine
                        nd.sync_info = mybir.SyncInfo(on_wait=[w], on_update=[])
                        outl.append(nd)
                    si.on_wait = [waits[-1]]
                    changed = True
                outl.append(inst)
            if changed:
                blk.instructions = outl
    return nc


def kernel(x0, x1, eca_w, conv_w, bn_gamma, bn_beta, bn_mean, bn_var):
    x0 = np.asarray(x0, np.float32).reshape(B, C, NPIX)
    x1 = np.asarray(x1, np.float32).reshape(B, C, NPIX)
    cst = host_consts(conv_w, bn_gamma, bn_beta, bn_mean, bn_var, eca_w)
    nc = _split_multiwait_drains(build_nc())
    in_maps = []
    for c in range(NCORES):
        m = dict(cst)
        m["x0"] = np.ascontiguousarray(x0[c * SPC:(c + 1) * SPC])
        m["x1"] = np.ascontiguousarray(x1[c * SPC:(c + 1) * SPC])
        in_maps.append(m)
    res = run_bass_kernel_spmd(nc, in_maps, list(range(NCORES)), trace=False)
    out = np.concatenate([res.results[c]["out"] for c in range(NCORES)], axis=0)
    return out.reshape(B, 2 * C, H, W)


# revision 3
# speedup vs baseline: 443.7003x; 1.1404x over previous
"""Trainium2 Bass kernel for nn_MFF_38809324487316 (topk_masking).

Pure data parallel: batch dim 16 -> 8 cores x 2 samples; the tiny ECA/conv/BN
params are replicated (folded into one packed constant block per core).

Per sample, the whole top-k gather / mean / 1x1-conv pipeline is one
data-dependent [256,256] @ [256,6400] matmul:
  rows   0..127 : G            -> x1[pos_idx]            (tmp1 gather rows)
  row    128    : negmask/128  -> mean of negatives      (tmp1 mean row)
  rows 129..255 : W_pos @ G + outer(w_last, negmask/128) (the 1x1 conv)
followed by BN + LeakyReLU (ACT affine + one max-based VE op; the mean row
passes through via per-partition alpha[0]=1) and a +x1 add.

The data-dependent 0/1 matrices are built on-device from the ECA channel
scores with no sort and no data-dependent control flow:
  - channel scores y for BOTH samples via two accumulating PE matmuls
    against a host-built banded matrix (GAP + ECA conv fused; sigmoid
    dropped - it is monotone so the ranking is unchanged),
  - ranks for all 4 (sample, half) pairs via fused compare+row-sum
    (tensor_scalar with accum_out),
  - gather matrix G via iota == position equality.

All big transfers ride ONE HWDGE queue (nc.sync) - measured faster on HW
than splitting across queues; the x0 passthrough is DRAM->DRAM after each
sample's stores. Per-iteration HW time ~178 us/core vs a measured pure-DMA
floor of ~165 us for the same traffic (52.4 MB HBM per core per iteration).
"""

import sys

sys.path.insert(0, "/opt/trn_rl_repo")

import numpy as np

import concourse.bass as bass
import concourse.tile as tile
from concourse import mybir
from concourse.bass_utils import run_bass_kernel_spmd

B, C, H, W = 16, 256, 80, 80
HALF = C // 2           # 128
NPIX = H * W            # 6400
NCORES = 8
SPC = B // NCORES       # 2 samples per core
NT = 512                # matmul n-tile (one PSUM bank of f32)
GRP = 1024              # epilogue group (2 PSUM banks)
BN_EPS = 1e-5
F32 = mybir.dt.float32
BF16 = mybir.dt.bfloat16

# cblk column offsets
O_ID = 0
O_TRI = 128
O_ONES = 256
O_IOTA = 384
O_B0 = 512
O_B1 = 768
O_WPT = 1024
O_WLB = 1151
O_BNA = 1278
O_BNB = 1279
O_ALP = 1280
O_SEL = 1281
CBLK_W = 1537


def host_consts(conv_w, bn_gamma, bn_beta, bn_mean, bn_var, eca_w):
    w = np.asarray(eca_w, np.float64).reshape(5)
    conv_w = np.asarray(conv_w, np.float64)          # [127, 129]
    id128 = np.eye(HALF)
    tri = np.triu(np.ones((HALF, HALF)), 1)          # tri[k, j] = 1 iff k < j
    ones = np.ones((HALF, HALF))
    iota = np.tile(np.arange(HALF, dtype=np.float64), (HALF, 1))
    Bm = np.zeros((2, HALF, C))
    for h in range(2):
        for k in range(HALF):
            c = h * HALF + k
            for t in range(5):
                cp = c - t + 2
                if 0 <= cp < C:
                    Bm[h, k, cp] = w[t]
    wposT = conv_w[:, :HALF].T                        # [128, 127]
    wlastb = np.tile(conv_w[:, HALF][None, :], (HALF, 1))
    a = np.asarray(bn_gamma, np.float64) / np.sqrt(
        np.asarray(bn_var, np.float64) + BN_EPS)
    bnA = np.zeros((HALF, 1)); bnA[0, 0] = 1.0; bnA[1:, 0] = a
    bnB = np.zeros((HALF, 1))
    bnB[1:, 0] = (np.asarray(bn_beta, np.float64)
                  - np.asarray(bn_mean, np.float64) * a)
    alpha = np.full((HALF, 1), 0.1); alpha[0, 0] = 1.0
    sel = np.zeros((HALF, 2 * HALF))
    sel[0, 0:HALF] = 1.0
    sel[1, HALF:2 * HALF] = 1.0
    cblk = np.concatenate(
        [id128, tri, ones, iota, Bm[0], Bm[1], wposT, wlastb, bnA, bnB, alpha,
         sel], axis=1).astype(np.float32)
    assert cblk.shape == (HALF, CBLK_W)
    import ml_dtypes
    cbf = np.concatenate([id128, wposT], axis=1).astype(ml_dtypes.bfloat16)
    return {"cblk": cblk, "cbf": cbf}


def build_nc(reps=1, npix=NPIX, nsamp=SPC, mh0_engine="vector"):
    nc = bass.Bass("TRN2", target_bir_lowering=False, debug=False)

    x0 = nc.dram_tensor("x0", [nsamp, C, npix], F32, kind="ExternalInput").ap()
    x1 = nc.dram_tensor("x1", [nsamp, C, npix], F32, kind="ExternalInput").ap()
    cbd = nc.dram_tensor("cblk", [HALF, CBLK_W], F32, kind="ExternalInput").ap()
    cbfd = nc.dram_tensor("cbf", [HALF, 255], BF16, kind="ExternalInput").ap()
    out = nc.dram_tensor("out", [nsamp, 2 * C, npix], F32,
                         kind="ExternalOutput").ap()

    AL = mybir.AluOpType
    from contextlib import ExitStack
    with tile.TileContext(nc) as tc, ExitStack() as st:
        consts = st.enter_context(tc.tile_pool(name="consts", bufs=1))
        xin = st.enter_context(tc.tile_pool(name="xin", bufs=1))
        lhp = st.enter_context(tc.tile_pool(name="lhp", bufs=1))
        misc = st.enter_context(tc.tile_pool(name="misc", bufs=1))
        epi = st.enter_context(tc.tile_pool(name="epi", bufs=2))
        obp = st.enter_context(tc.tile_pool(name="obp", bufs=2))
        prk = st.enter_context(tc.tile_pool(name="prk", bufs=3, space="PSUM"))
        pbig = st.enter_context(tc.tile_pool(name="pbig", bufs=2, space="PSUM"))

        cb = consts.tile([HALF, CBLK_W], F32)
        nc.sync.dma_start(out=cb, in_=cbd)
        cbf = consts.tile([HALF, 255], BF16)
        nc.sync.dma_start(out=cbf, in_=cbfd)
        c_id16 = cbf[:, 0:128]
        c_wposT16 = cbf[:, 128:255]
        c_id = cb[:, O_ID:O_ID + 128]
        c_id2 = cb[0:2, O_ID:O_ID + 2]
        c_tri = cb[:, O_TRI:O_TRI + 128]
        c_ones = cb[:, O_ONES:O_ONES + 128]
        c_iota = cb[:, O_IOTA:O_IOTA + 128]
        c_B = [cb[:, O_B0:O_B0 + C], cb[:, O_B1:O_B1 + C]]
        c_wposT = cb[:, O_WPT:O_WPT + 127]
        c_wlastb = cb[:, O_WLB:O_WLB + 127]
        c_bnA = cb[:, O_BNA:O_BNA + 1]
        c_bnB = cb[:, O_BNB:O_BNB + 1]
        c_alpha = cb[:, O_ALP:O_ALP + 1]

        for rep in range(reps):
            # ---- loads (f32 bounce) + fused bf16 cast + channel sums ----
            X = [[None, None] for _ in range(nsamp)]
            SM = misc.tile([HALF, 4], F32, tag="SM")
            for s in range(nsamp):
                for h in range(2):
                    t = xin.tile([HALF, npix], F32, tag="xt", bufs=2)
                    nc.sync.dma_start(out=t,
                                      in_=x1[s, h * HALF:(h + 1) * HALF, :])
                    xb = xin.tile([HALF, npix], BF16, tag=f"xb_{s}_{h}")
                    nc.vector.tensor_scalar(
                        out=xb, in0=t, scalar1=1.0, scalar2=None,
                        op0=AL.mult, op1=AL.add,
                        accum_out=SM[:, h * 2 + s:h * 2 + s + 1])
                    X[s][h] = xb

            # ---- scores y (GAP+ECA fused): Y2 [2, 256] = sum_h SM_h^T @ B_h ----
            Y2 = prk.tile([2, C], F32, tag="mp")
            nc.tensor.matmul(Y2, SM[:, 0:2], c_B[0], start=True, stop=False)
            nc.tensor.matmul(Y2, SM[:, 2:4], c_B[1], start=False, stop=True)
            y_sb = misc.tile([2, C], F32, tag="ysb")
            nc.vector.tensor_copy(out=y_sb, in_=Y2)

            # ---- yT [128, 4]: y with channel-as-partition ----
            pyT = prk.tile([HALF, 4], F32, tag="mp")
            for h in range(2):
                nc.tensor.matmul(pyT[:, h * 2:h * 2 + 2],
                                 y_sb[:, h * HALF:(h + 1) * HALF], c_id2,
                                 start=True, stop=True)
            ycT = misc.tile([HALF, 4], F32, tag="ycT")
            nc.vector.tensor_copy(out=ycT, in_=pyT)

            # ---- broadcast y along partitions: pbY [128, 512] ----
            pbY = prk.tile([HALF, 2 * C], F32, tag="mp")
            for s in range(nsamp):
                nc.tensor.matmul(pbY[:, s * C:(s + 1) * C],
                                 cb[0:2, O_SEL + s * HALF:O_SEL + (s + 1) * HALF],
                                 y_sb, start=True, stop=True)

            # ---- ranks RD[:, h*2+s] = #{c' : y[c'] > y[c]} ----
            RD = misc.tile([HALF, 4], F32, tag="RD")
            for s in range(nsamp):
                for h in range(2):
                    junk = misc.tile([HALF, C], F32, tag="junk", bufs=2)
                    nc.vector.tensor_scalar(
                        out=junk, in0=pbY[:, s * C:(s + 1) * C],
                        scalar1=ycT[:, h * 2 + s:h * 2 + s + 1], scalar2=None,
                        op0=AL.is_gt, op1=AL.add,
                        accum_out=RD[:, h * 2 + s:h * 2 + s + 1])

            # ---- masks ----
            M = misc.tile([HALF, 4], F32, tag="M")
            ND = misc.tile([HALF, 4], F32, tag="ND")
            nc.vector.tensor_scalar(out=M, in0=RD, scalar1=float(HALF),
                                    scalar2=None, op0=AL.is_lt)
            nc.vector.tensor_scalar(out=ND, in0=RD, scalar1=float(HALF),
                                    scalar2=1.0 / HALF, op0=AL.is_ge,
                                    op1=AL.mult)

            # ---- positions P; RP = 32768*ND + P ----
            P = prk.tile([HALF, 4], F32, tag="mp")
            nc.tensor.matmul(P[:, 0:2], c_tri, M[:, 0:2], start=True, stop=True)
            nc.tensor.matmul(P[:, 2:4], c_tri, M[:, 2:4], start=True, stop=False)
            nc.tensor.matmul(P[:, 2:4], c_ones, M[:, 0:2], start=False,
                             stop=True)
            RP = misc.tile([HALF, 4], F32, tag="RP")
            nc.vector.scalar_tensor_tensor(out=RP, in0=ND, scalar=32768.0,
                                           in1=P, op0=AL.mult, op1=AL.add)

            # ---- G columns + negdiv column of LHS ----
            LHS = [[None, None] for _ in range(nsamp)]
            for s in range(nsamp):
                for h in range(2):
                    lh = lhp.tile([HALF, C], BF16, tag=f"lh_{s}_{h}")
                    LHS[s][h] = lh
                    nc.vector.tensor_scalar(
                        out=lh[:, 0:HALF], in0=c_iota,
                        scalar1=RP[:, h * 2 + s:h * 2 + s + 1], scalar2=None,
                        op0=AL.is_equal)
                    nc.vector.tensor_copy(out=lh[:, HALF:HALF + 1],
                                          in_=ND[:, h * 2 + s:h * 2 + s + 1])

            # ---- W columns: transpose G, multiply by W_pos^T, assemble ----
            sh_pairs = [(s, h) for s in range(nsamp) for h in range(2)]
            pgm = prk.tile([HALF, 4 * HALF], F32, tag="mp")
            for i, (s, h) in enumerate(sh_pairs):
                nc.tensor.matmul(pgm[:, i * HALF:(i + 1) * HALF],
                                 LHS[s][h][:, 0:HALF], c_id16,
                                 start=True, stop=True)
            gm_all = misc.tile([HALF, 4 * HALF], BF16, tag="gm")
            nc.vector.tensor_copy(out=gm_all, in_=pgm)
            pwg = prk.tile([HALF, 4 * HALF], F32, tag="mp")
            for i, (s, h) in enumerate(sh_pairs):
                nc.tensor.matmul(pwg[:, i * HALF:i * HALF + 127],
                                 gm_all[:, i * HALF:(i + 1) * HALF], c_wposT16,
                                 start=True, stop=True)
            for i, (s, h) in enumerate(sh_pairs):
                nc.vector.scalar_tensor_tensor(
                    out=LHS[s][h][:, HALF + 1:C], in0=c_wlastb,
                    scalar=ND[:, h * 2 + s:h * 2 + s + 1],
                    in1=pwg[:, i * HALF:i * HALF + 127],
                    op0=AL.mult, op1=AL.add)

            # ---- big matmuls + epilogue + stores ----
            grps = []
            g0 = 0
            while g0 < npix:
                grps.append((g0, min(GRP, npix - g0)))
                g0 += GRP
            for s in range(nsamp):
                for mh in range(2):
                    ob = obp.tile([HALF, npix], F32, tag="ob")
                    for (g0, gsz) in grps:
                        ps = pbig.tile([HALF, GRP], F32, tag="pb")
                        n0 = 0
                        while n0 < gsz:
                            nsz = min(NT, gsz - n0)
                            for h in range(2):
                                nc.tensor.matmul(
                                    ps[:, n0:n0 + nsz],
                                    LHS[s][h][:, mh * HALF:(mh + 1) * HALF],
                                    X[s][h][:, g0 + n0:g0 + n0 + nsz],
                                    start=(h == 0), stop=(h == 1))
                            n0 += nsz
                        if mh == 0:
                            getattr(nc, mh0_engine).tensor_add(
                                out=ob[:, g0:g0 + gsz], in0=ps[:, :gsz],
                                in1=X[s][0][:, g0:g0 + gsz])
                        else:
                            q = epi.tile([HALF, GRP], F32, tag="q")
                            nc.scalar.activation(
                                out=q[:, :gsz], in_=ps[:, :gsz],
                                func=mybir.ActivationFunctionType.Identity,
                                bias=c_bnB, scale=c_bnA)
                            m = epi.tile([HALF, GRP], F32, tag="m")
                            nc.vector.scalar_tensor_tensor(
                                out=m[:, :gsz], in0=q[:, :gsz], scalar=c_alpha,
                                in1=q[:, :gsz], op0=AL.mult, op1=AL.max)
                            nc.vector.tensor_add(out=ob[:, g0:g0 + gsz],
                                                 in0=m[:, :gsz],
                                                 in1=X[s][1][:, g0:g0 + gsz])
                    nc.sync.dma_start(
                        out=out[s, C + mh * HALF:C + (mh + 1) * HALF, :],
                        in_=ob)
                # x0 passthrough DRAM->DRAM, after this sample's stores
                nc.sync.dma_start(out=out[s, 0:C, :], in_=x0[s, :, :])
    return nc


def _split_multiwait_drains(nc):
    """This container's walrus rejects >1 sync-wait on one instruction -
    split Tile's kernel-tail multi-wait Drains into single-wait chains."""
    for fn in nc.m.functions:
        for blk in fn.blocks:
            insts = list(blk.instructions)
            changed = False
            outl = []
            for inst in insts:
                si = getattr(inst, "sync_info", None)
                waits = list(si.on_wait) if (si and si.on_wait) else []
                if len(waits) > 1:
                    for j, w in enumerate(waits[:-1]):
                        nd = mybir.InstEventSemaphore(
                            name=f"{inst.name}-sw{j}", ins=[], outs=[])
                        nd.engine = inst.engine
                        nd.sync_info = mybir.SyncInfo(on_wait=[w], on_update=[])
                        outl.append(nd)
                    si.on_wait = [waits[-1]]
                    changed = True
                outl.append(inst)
            if changed:
                blk.instructions = outl
    return nc


def kernel(x0, x1, eca_w, conv_w, bn_gamma, bn_beta, bn_mean, bn_var):
    x0 = np.asarray(x0, np.float32).reshape(B, C, NPIX)
    x1 = np.asarray(x1, np.float32).reshape(B, C, NPIX)
    cst = host_consts(conv_w, bn_gamma, bn_beta, bn_mean, bn_var, eca_w)
    nc = _split_multiwait_drains(build_nc())
    in_maps = []
    for c in range(NCORES):
        m = dict(cst)
        m["x0"] = np.ascontiguousarray(x0[c * SPC:(c + 1) * SPC])
        m["x1"] = np.ascontiguousarray(x1[c * SPC:(c + 1) * SPC])
        in_maps.append(m)
    res = run_bass_kernel_spmd(nc, in_maps, list(range(NCORES)), trace=False)
    out = np.concatenate([res.results[c]["out"] for c in range(NCORES)], axis=0)
    return out.reshape(B, 2 * C, H, W)
